# revision 23
# baseline (speedup 1.0000x reference)
"""Quantized dense MLP kernel for 8 Trainium2 NeuronCores.

Problem: out = relu(inputs @ ((w_int8 - zero_point) * scale) + b)
  inputs [8192, 2048] f32, w_quantized [2048, 8192] int8,
  scale/zero_point f32 scalars, b [8192] f32 -> out [8192, 8192] f32.

Strategy:
- Data-parallel: shard rows of `inputs` across 8 cores (1024 rows each).
- Zero-point folding: w_int = w_int8 - zero_point (zero_point = -3.0) is a
  small integer, exactly representable in bf16/f32. The scale and bias are
  applied on the ScalarEngine in f32: out = Relu(scale * acc + b).
- The PE matmul runs in float32r (TF32-like, full bf16 throughput at
  N=512): exact weights, ~1e-4 rel err from the reduced-precision x
  streaming. (MM_DTYPE="bfloat16" variant: ~6% faster, ~1.7e-3 rel err.)
- On device each core computes outT[j, i] = sum_k w_int[k, j] * xT[k, i]
  (w tile stationary, xT moving), so the bias b_j is a natural
  per-partition activation bias. The host transposes each core's outT
  back and stacks.
- Per core: x^T stays SBUF-resident; weights stream as G large 2D DMAs
  (the sync engine pays ~600 ns issue per DMA regardless of size, so few
  big transfers); group 0 lands piecewise with x on a parallel HW queue
  so the PE starts within ~16 us.
"""

import numpy as np
import ml_dtypes

import concourse.bass as bass
import concourse.mybir as mybir
import concourse.tile as tile
from concourse import bacc
from concourse.bass_utils import run_bass_kernel_spmd

BF16 = ml_dtypes.bfloat16

# Full problem dims (hardcoded per harness contract).
ROWS, D_IN, UNITS = 8192, 2048, 8192
N_CORES = 8
ROWS_C = ROWS // N_CORES  # rows per core

P = 128         # SBUF partitions
N_SLICE = 512   # moving free dim per matmul (one PSUM bank of f32)


def build_nc(scale: float, d_in: int = D_IN, units: int = UNITS,
             rows_c: int = ROWS_C, mm_dtype: str = "bfloat16"):
    """Build + compile the per-core Bass program (SPMD, identical on all
    cores).

    DRAM inputs (per core):
      xt [KT, 128, rows_c]  bf16 : x-shard transposed, k-tiled
      w  [G, 128, JG*KT*128] bf16 : w_int, G groups of JG j-tiles;
            w[g, p, jtl*KT*128 + kt*128 + f] = w_int[kt*128+p, (g*JG+jtl)*128+f]
            so each group is one [128 x JG*KT*128] 2D DMA (16KB/partition).
      bt [128, JT]          f32  : bias, bt[p, jt] = b[jt*128 + p]
    DRAM output:
      o  [JT, 128, rows_c]  f32  : outT tiles, o[jt, p, i] = outT[jt*128+p, i]
    """
    KT = d_in // P
    JT = units // P
    NS = rows_c // N_SLICE
    # mm_dtype: "bfloat16" | "float32r" | "mixed" (bf16 weights, f32r x)
    w_dt = mybir.dt.bfloat16 if mm_dtype in ("bfloat16", "mixed") \
        else mybir.dt.float32r
    x_dt = mybir.dt.bfloat16 if mm_dtype == "bfloat16" else mybir.dt.float32r
    # j-tiles per weight DMA group (f32r tiles are 2x the bytes; keep SBUF fit)
    JG = min(8 if w_dt == mybir.dt.bfloat16 else 4, JT)
    G = JT // JG
    WBUFS = 3 if w_dt == mybir.dt.bfloat16 else 2

    nc = bacc.Bacc(None, target_bir_lowering=False)
    xt = nc.dram_tensor("xt", [KT, P, rows_c], x_dt, kind="ExternalInput")
    w = nc.dram_tensor("w", [G, P, JG * KT * P], w_dt, kind="ExternalInput")
    bt = nc.dram_tensor("bt", [P, JT], mybir.dt.float32, kind="ExternalInput")
    o = nc.dram_tensor("o", [JT, P, rows_c], mybir.dt.float32,
                       kind="ExternalOutput")

    with tile.TileContext(nc) as tc:
        with (
            tc.tile_pool(name="xpool", bufs=1) as xpool,
            tc.tile_pool(name="bpool", bufs=1) as bpool,
            tc.tile_pool(name="wpool", bufs=WBUFS) as wpool,
            tc.tile_pool(name="opool", bufs=3) as opool,
            tc.tile_pool(name="pspool", bufs=4, space="PSUM") as pspool,
        ):
            # Prologue: w group 0 lands piecewise on the SP HW queue so the
            # first j-tile's weights arrive early, while x k-tiles stream in
            # parallel on the Activation engine's HW queue. The PE starts as
            # soon as w[jtl=0] + x[kt=0] are in (slice-level deps).
            wsbs = [wpool.tile([P, JG * KT * P], w_dt,
                               tag="wsb", name=f"wsb{g}") for g in range(G)]
            for jtl in range(JG):
                nc.sync.dma_start(
                    out=wsbs[0][:, jtl * KT * P:(jtl + 1) * KT * P],
                    in_=w[0, :, jtl * KT * P:(jtl + 1) * KT * P],
                )

            # Resident activations: all k-tiles of xT, side by side. Loaded
            # in n-slice halves, all n=0 halves first: the first psum groups
            # read only the n=0 columns, so they start ~2x sooner.
            xsb = xpool.tile([P, KT * rows_c], x_dt)
            for n in range(NS):
                for kt in range(KT):
                    nc.scalar.dma_start(
                        out=xsb[:, kt * rows_c + n * N_SLICE:
                                   kt * rows_c + (n + 1) * N_SLICE],
                        in_=xt[kt][:, n * N_SLICE:(n + 1) * N_SLICE],
                    )
            bsb = bpool.tile([P, JT], mybir.dt.float32)
            nc.scalar.dma_start(out=bsb[:, :], in_=bt[:, :])

            for g in range(G):
                wsb = wsbs[g]
                if g > 0:
                    # Prefetch on the Activation HW queue, queued behind the
                    # x tiles: keeps the early weight groups from stealing
                    # HBM bandwidth from the critical x stream, while the SP
                    # queue carries only w group 0 + output writes.
                    nc.scalar.dma_start(out=wsb[:, :], in_=w[g])
                for jtl in range(JG):
                    jt = g * JG + jtl
                    ob = opool.tile([P, rows_c], mybir.dt.float32)
                    for n in range(NS):
                        ps = pspool.tile([P, N_SLICE], mybir.dt.float32)
                        for kt in range(KT):
                            wof = jtl * KT * P + kt * P
                            nc.tensor.matmul(
                                ps[:, :],
                                wsb[:, wof:wof + P],
                                xsb[:, kt * rows_c + n * N_SLICE:
                                       kt * rows_c + (n + 1) * N_SLICE],
                                start=(kt == 0),
                                stop=(kt == KT - 1),
                            )
                        nc.scalar.activation(
                            ob[:, n * N_SLICE:(n + 1) * N_SLICE],
                            ps[:, :],
                            mybir.ActivationFunctionType.Relu,
                            bias=bsb[:, jt:jt + 1],
                            scale=float(scale),
                        )
                    nc.sync.dma_start(out=o[jt], in_=ob[:, :])

    nc.compile()
    return nc


def prep_w(w_int, d_in: int = None, units: int = None,
           mm_dtype: str = "bfloat16"):
    """[d_in, units] -> [G, 128, JG*KT*128]; see build_nc docstring."""
    d_in = d_in or w_int.shape[0]
    units = units or w_int.shape[1]
    KT, JT = d_in // P, units // P
    JG = min(8 if mm_dtype in ("bfloat16", "mixed") else 4, JT)
    G = JT // JG
    return np.ascontiguousarray(
        w_int.reshape(KT, P, G, JG, P)        # [kt, p, g, jtl, f]
             .transpose(2, 1, 3, 0, 4)        # [g, p, jtl, kt, f]
             .reshape(G, P, JG * KT * P)
    )


_NC_CACHE: dict = {}


MM_DTYPE = "float32r"   # "bfloat16" | "float32r" | "mixed"


def _get_nc(scale: float):
    key = (round(float(scale), 12), MM_DTYPE)
    if key not in _NC_CACHE:
        _NC_CACHE[key] = build_nc(float(scale), mm_dtype=MM_DTYPE)
    return _NC_CACHE[key]


def kernel(inputs, w_quantized, quantized_scale, zero_point, b):
    scale = float(np.asarray(quantized_scale))
    zp = float(np.asarray(zero_point))

    # Exact integer weights in bf16 (w - zp with zp = -3.0 stays a small
    # integer; bf16 represents integers up to 256 exactly).
    w_int = (np.asarray(w_quantized).astype(np.float32) - zp)
    if MM_DTYPE in ("bfloat16", "mixed"):
        w_int = w_int.astype(BF16)
    w_tiled = prep_w(w_int, mm_dtype=MM_DTYPE)

    bt = np.ascontiguousarray(
        np.asarray(b).astype(np.float32).reshape(UNITS // P, P).T
    )

    x_bf = np.asarray(inputs).astype(np.float32)
    if MM_DTYPE == "bfloat16":
        x_bf = x_bf.astype(BF16)

    in_maps = []
    for c in range(N_CORES):
        shard = x_bf[c * ROWS_C:(c + 1) * ROWS_C, :]          # [1024, 2048]
        xt_c = np.ascontiguousarray(shard.T).reshape(D_IN // P, P, ROWS_C)
        in_maps.append({"xt": xt_c, "w": w_tiled, "bt": bt})

    nc = _get_nc(scale)
    results = run_bass_kernel_spmd(nc, in_maps, core_ids=list(range(N_CORES)))
    global _LAST_RESULTS
    _LAST_RESULTS = results

    out = np.empty((ROWS, UNITS), dtype=np.float32)
    for c in range(N_CORES):
        outT = results.results[c]["o"].reshape(UNITS, ROWS_C)
        out[c * ROWS_C:(c + 1) * ROWS_C, :] = outT.T
    return out


# revision 26
# speedup vs baseline: 1.0065x; 1.0065x over previous
"""Quantized dense MLP kernel for 8 Trainium2 NeuronCores.

Problem: out = relu(inputs @ ((w_int8 - zero_point) * scale) + b)
  inputs [8192, 2048] f32, w_quantized [2048, 8192] int8,
  scale/zero_point f32 scalars, b [8192] f32 -> out [8192, 8192] f32.

Strategy:
- Data-parallel: shard rows of `inputs` across 8 cores (1024 rows each).
- Zero-point folding: w_int = w_int8 - zero_point (zero_point = -3.0) is a
  small integer, exactly representable in bf16/f32. The scale and bias are
  applied on the ScalarEngine in f32: out = Relu(scale * acc + b).
- The PE matmul runs in float32r (TF32-like, full bf16 throughput at
  N=512): exact weights, ~1e-4 rel err from the reduced-precision x
  streaming. (MM_DTYPE="bfloat16" variant: ~6% faster, ~1.7e-3 rel err.)
- On device each core computes outT[j, i] = sum_k w_int[k, j] * xT[k, i]
  (w tile stationary, xT moving), so the bias b_j is a natural
  per-partition activation bias. The host transposes each core's outT
  back and stacks.
- Per core: x^T stays SBUF-resident; weights stream as G large 2D DMAs
  (the sync engine pays ~600 ns issue per DMA regardless of size, so few
  big transfers); group 0 lands piecewise with x on a parallel HW queue
  so the PE starts within ~16 us.
"""

import numpy as np
import ml_dtypes

import concourse.bass as bass
import concourse.mybir as mybir
import concourse.tile as tile
from concourse import bacc
from concourse.bass_utils import run_bass_kernel_spmd

BF16 = ml_dtypes.bfloat16

# Full problem dims (hardcoded per harness contract).
ROWS, D_IN, UNITS = 8192, 2048, 8192
N_CORES = 8
ROWS_C = ROWS // N_CORES  # rows per core

P = 128         # SBUF partitions
N_SLICE = 512   # moving free dim per matmul (one PSUM bank of f32)


def build_nc(scale: float, d_in: int = D_IN, units: int = UNITS,
             rows_c: int = ROWS_C, mm_dtype: str = "bfloat16"):
    """Build + compile the per-core Bass program (SPMD, identical on all
    cores).

    DRAM inputs (per core):
      xt [KT, 128, rows_c]  bf16 : x-shard transposed, k-tiled
      w  [G, 128, JG*KT*128] bf16 : w_int, G groups of JG j-tiles;
            w[g, p, jtl*KT*128 + kt*128 + f] = w_int[kt*128+p, (g*JG+jtl)*128+f]
            so each group is one [128 x JG*KT*128] 2D DMA (16KB/partition).
      bt [128, JT]          f32  : bias, bt[p, jt] = b[jt*128 + p]
    DRAM output:
      o  [JT, 128, rows_c]  f32  : outT tiles, o[jt, p, i] = outT[jt*128+p, i]
    """
    KT = d_in // P
    JT = units // P
    NS = rows_c // N_SLICE
    # mm_dtype: "bfloat16" | "float32r" | "mixed" (bf16 weights, f32r x)
    w_dt = mybir.dt.bfloat16 if mm_dtype in ("bfloat16", "mixed") \
        else mybir.dt.float32r
    x_dt = mybir.dt.bfloat16 if mm_dtype == "bfloat16" else mybir.dt.float32r
    # j-tiles per weight DMA group (f32r tiles are 2x the bytes; keep SBUF fit)
    JG = min(8 if w_dt == mybir.dt.bfloat16 else 4, JT)
    G = JT // JG
    WBUFS = 3 if w_dt == mybir.dt.bfloat16 else 2

    nc = bacc.Bacc(None, target_bir_lowering=False)
    xt = nc.dram_tensor("xt", [KT, P, rows_c], x_dt, kind="ExternalInput")
    w = nc.dram_tensor("w", [G, P, JG * KT * P], w_dt, kind="ExternalInput")
    bt = nc.dram_tensor("bt", [P, JT], mybir.dt.float32, kind="ExternalInput")
    o = nc.dram_tensor("o", [JT, P, rows_c], mybir.dt.float32,
                       kind="ExternalOutput")

    with tile.TileContext(nc) as tc:
        with (
            tc.tile_pool(name="xpool", bufs=1) as xpool,
            tc.tile_pool(name="bpool", bufs=1) as bpool,
            tc.tile_pool(name="wpool", bufs=WBUFS) as wpool,
            tc.tile_pool(name="opool", bufs=3) as opool,
            tc.tile_pool(name="pspool", bufs=4, space="PSUM") as pspool,
        ):
            # Prologue: w group 0 lands piecewise on the SP HW queue so the
            # first j-tile's weights arrive early, while x k-tiles stream in
            # parallel on the Activation engine's HW queue. The PE starts as
            # soon as w[jtl=0] + x[kt=0] are in (slice-level deps).
            wsbs = [wpool.tile([P, JG * KT * P], w_dt,
                               tag="wsb", name=f"wsb{g}") for g in range(G)]
            for jtl in range(JG):
                nc.sync.dma_start(
                    out=wsbs[0][:, jtl * KT * P:(jtl + 1) * KT * P],
                    in_=w[0, :, jtl * KT * P:(jtl + 1) * KT * P],
                )

            # Resident activations: all k-tiles of xT, side by side.
            # (Splitting these into n-halves for an earlier first psum group
            # was tried and measured ~8 us slower: the half-tile reads are
            # 2 KB strided segments vs 4 KB contiguous rows.)
            xsb = xpool.tile([P, KT * rows_c], x_dt)
            for kt in range(KT):
                nc.scalar.dma_start(
                    out=xsb[:, kt * rows_c:(kt + 1) * rows_c], in_=xt[kt]
                )
            bsb = bpool.tile([P, JT], mybir.dt.float32)
            nc.scalar.dma_start(out=bsb[:, :], in_=bt[:, :])

            for g in range(G):
                wsb = wsbs[g]
                if g > 0:
                    # Prefetch on the Activation HW queue, queued behind the
                    # x tiles: keeps the early weight groups from stealing
                    # HBM bandwidth from the critical x stream, while the SP
                    # queue carries only w group 0 + output writes.
                    nc.scalar.dma_start(out=wsb[:, :], in_=w[g])
                for jtl in range(JG):
                    jt = g * JG + jtl
                    ob = opool.tile([P, rows_c], mybir.dt.float32)
                    for n in range(NS):
                        ps = pspool.tile([P, N_SLICE], mybir.dt.float32)
                        for kt in range(KT):
                            wof = jtl * KT * P + kt * P
                            nc.tensor.matmul(
                                ps[:, :],
                                wsb[:, wof:wof + P],
                                xsb[:, kt * rows_c + n * N_SLICE:
                                       kt * rows_c + (n + 1) * N_SLICE],
                                start=(kt == 0),
                                stop=(kt == KT - 1),
                            )
                        nc.scalar.activation(
                            ob[:, n * N_SLICE:(n + 1) * N_SLICE],
                            ps[:, :],
                            mybir.ActivationFunctionType.Relu,
                            bias=bsb[:, jt:jt + 1],
                            scale=float(scale),
                        )
                    nc.sync.dma_start(out=o[jt], in_=ob[:, :])

    nc.compile()
    return nc


def prep_w(w_int, d_in: int = None, units: int = None,
           mm_dtype: str = "bfloat16"):
    """[d_in, units] -> [G, 128, JG*KT*128]; see build_nc docstring."""
    d_in = d_in or w_int.shape[0]
    units = units or w_int.shape[1]
    KT, JT = d_in // P, units // P
    JG = min(8 if mm_dtype in ("bfloat16", "mixed") else 4, JT)
    G = JT // JG
    return np.ascontiguousarray(
        w_int.reshape(KT, P, G, JG, P)        # [kt, p, g, jtl, f]
             .transpose(2, 1, 3, 0, 4)        # [g, p, jtl, kt, f]
             .reshape(G, P, JG * KT * P)
    )


_NC_CACHE: dict = {}


MM_DTYPE = "float32r"   # "bfloat16" | "float32r" | "mixed"


def _get_nc(scale: float):
    key = (round(float(scale), 12), MM_DTYPE)
    if key not in _NC_CACHE:
        _NC_CACHE[key] = build_nc(float(scale), mm_dtype=MM_DTYPE)
    return _NC_CACHE[key]


def kernel(inputs, w_quantized, quantized_scale, zero_point, b):
    scale = float(np.asarray(quantized_scale))
    zp = float(np.asarray(zero_point))

    # Exact integer weights in bf16 (w - zp with zp = -3.0 stays a small
    # integer; bf16 represents integers up to 256 exactly).
    w_int = (np.asarray(w_quantized).astype(np.float32) - zp)
    if MM_DTYPE in ("bfloat16", "mixed"):
        w_int = w_int.astype(BF16)
    w_tiled = prep_w(w_int, mm_dtype=MM_DTYPE)

    bt = np.ascontiguousarray(
        np.asarray(b).astype(np.float32).reshape(UNITS // P, P).T
    )

    x_bf = np.asarray(inputs).astype(np.float32)
    if MM_DTYPE == "bfloat16":
        x_bf = x_bf.astype(BF16)

    in_maps = []
    for c in range(N_CORES):
        shard = x_bf[c * ROWS_C:(c + 1) * ROWS_C, :]          # [1024, 2048]
        xt_c = np.ascontiguousarray(shard.T).reshape(D_IN // P, P, ROWS_C)
        in_maps.append({"xt": xt_c, "w": w_tiled, "bt": bt})

    nc = _get_nc(scale)
    results = run_bass_kernel_spmd(nc, in_maps, core_ids=list(range(N_CORES)))
    global _LAST_RESULTS
    _LAST_RESULTS = results

    out = np.empty((ROWS, UNITS), dtype=np.float32)
    for c in range(N_CORES):
        outT = results.results[c]["o"].reshape(UNITS, ROWS_C)
        out[c * ROWS_C:(c + 1) * ROWS_C, :] = outT.T
    return out
